# revision 13
# baseline (speedup 1.0000x reference)
"""Trainium2 Bass kernel for nn_D_loss_67551245631962.

Computes: 0.8 * sum(WMA5(target_angle - pred_angle)^2) + 0.2 * sum((target_class - pred_class)^2)
where WMA5 is a 5-tap [0.05, 0.1, 0.7, 0.1, 0.05] correlation with 2-zero padding per side.

Strategy (pure data parallelism over batch dim B=2048 across 8 cores, 256 rows/core):
  - Inputs cast to fp16 on the host (same numerics as the original on-chip
    cast-DMA pipeline, ~1e-5 end-to-end) -> per-core HBM read 8.4 MB.
  - Row-group merge: the 256 rows/core are laid out [128 partitions, 2 row
    segments] (DRAM tensor declared [128, 2, T]; partition p holds rows
    2p, 2p+1). Every DVE op covers BOTH segments as one 3D-AP instruction,
    halving instruction/semaphore overhead vs the 2x128-group layout.
  - All loads on the SP HWDGE ring (measured 300+ GB/s interleaved; keeps
    ACT free of emissions, GpSimd free). Chunks [512,2048,2048,2048,1024,
    256,256]: small lead (compute starts ~5us in), small tail.
  - s = 14*d2 + 2*u + v (u = d1+d3, v = d0+d4) = wma/0.05, exact 5 taps.
    DVE: dbf = ta-pa, u, v (GpSimd compute is NOT used: concurrent Q7
    tensor ops degrade DVE throughput ~4x - measured).
    j=0..4 (93.75%): PE psum = 14I@d2 + 2I@u + I@v per (chunk, seg);
      ACT squares psum with accum_out.
    j=5,6 (the tail): all-DVE STT chain r2=2u+v, s=14*d2+r2,
      sq=s*s with accum_out - no cross-engine hops after the last load.
  - Halo memsets + identity stationaries built on GpSimd before loads.
  - Host sums the 8 cores' [128, 16] partials in float64, scales by
    0.8*0.05^2 (angle) and 0.2 (class).
  Engine budget/core (measured rates): DVE ~42us, PE ~43us, ACT ~27us,
  sync ~19us, DMA ~24us. Bottleneck: DVE/PE ~43us.
"""

import os
import sys

os.environ.setdefault("TILE_SCHEDULER", "asap")

for _p in ("/opt/trn_rl_repo",):
    if os.path.isdir(_p) and _p not in sys.path:
        sys.path.insert(0, _p)

from contextlib import ExitStack

import numpy as np

import concourse.bass as bass
import concourse.tile as tile
from concourse import bacc, mybir
from concourse.bass_utils import run_bass_kernel_spmd

N_CORES = 8
B, T = 2048, 8192
RPC = B // N_CORES  # rows per core = 256
SEG = 2             # row segments per partition (rows 2p, 2p+1)

LW = [512, 2048, 2048, 2048, 1024, 256, 256]
assert sum(LW) == T
LSTART = [sum(LW[:j]) for j in range(len(LW))]
NL = len(LW)
N_PE = 5      # chunks 0..4 -> PE path; 5,6 -> all-DVE tail
NACC = N_PE * SEG + (NL - N_PE) + 1  # PE cols + tail cols + class col = 13
CH = 512

W4 = 0.05
DT16 = mybir.dt.float16


def build_nc():
    nc = bacc.Bacc("TRN2")
    dt = mybir.dt
    ta = nc.dram_tensor("target_angle", [128, SEG, T], DT16, kind="ExternalInput")
    pa = nc.dram_tensor("pred_angle", [128, SEG, T], DT16, kind="ExternalInput")
    tcl = nc.dram_tensor("target_class", [128, SEG, 3], dt.float32, kind="ExternalInput")
    pcl = nc.dram_tensor("pred_class", [128, SEG, 3], dt.float32, kind="ExternalInput")
    out = nc.dram_tensor("out", [128, NACC], dt.float32, kind="ExternalOutput")

    AF = mybir.ActivationFunctionType
    OP = mybir.AluOpType

    def lgeom(j):
        c0, w = LSTART[j], LW[j]
        lo, hi = c0 - 2, c0 + w + 2
        dst_lo, dst_hi = 0, w + 4
        if lo < 0:
            dst_lo, lo = 2, 0
        if hi > T:
            dst_hi, hi = w + 2, T
        return lo, hi, dst_lo, dst_hi

    with tile.TileContext(nc) as tc, ExitStack() as ctx:
        pool = ctx.enter_context(tc.tile_pool(name="main", bufs=1))
        ppool = ctx.enter_context(tc.tile_pool(name="ps", bufs=2, space="PSUM"))

        accA = pool.tile([128, N_PE * SEG], dt.float32, tag="accA", bufs=1)
        accT = pool.tile([128, NL - N_PE], dt.float32, tag="accT", bufs=1)
        accC = pool.tile([128, 1], dt.float32, tag="accC", bufs=1)

        def make_diag(scale, name):
            m = pool.tile([128, 128], DT16, tag="diag", bufs=6, name=f"m_{name}")
            nc.gpsimd.memset(m[:], scale)
            s = pool.tile([128, 128], DT16, tag="diag", bufs=6, name=f"id_{name}")
            nc.gpsimd.affine_select(
                s[:], m[:], [[1, 128]], OP.is_equal, 0.0,
                base=0, channel_multiplier=-1,
            )
            return s

        id14 = make_diag(14.0, "w14")
        id2 = make_diag(2.0, "w2")
        id1 = make_diag(1.0, "w1")

        tas = [None] * NL
        pas = [None] * NL
        for j in range(NL):
            wid = LW[j] + 4
            tas[j] = pool.tile([128, SEG, wid], DT16, tag=f"ta{j}", bufs=1,
                               name=f"ta_{j}")
            pas[j] = pool.tile([128, SEG, wid], DT16, tag=f"pa{j}", bufs=1,
                               name=f"pa_{j}")

        # halo zeros on GpSimd (it is idle before loads; no DVE interference)
        wlast = LW[NL - 1]
        for tl in (tas[0], pas[0]):
            nc.gpsimd.memset(tl[:, :, 0:2], 0.0)
        for tl in (tas[NL - 1], pas[NL - 1]):
            nc.gpsimd.memset(tl[:, :, wlast + 2 : wlast + 4], 0.0)

        # all loads on the SP HWDGE ring; ta then pa per chunk; small lead,
        # small tail
        ctl = cpl = None
        for j in range(NL):
            lo, hi, dst_lo, dst_hi = lgeom(j)
            nc.sync.dma_start(tas[j][:, :, dst_lo:dst_hi], ta[:, :, lo:hi])
            nc.sync.dma_start(pas[j][:, :, dst_lo:dst_hi], pa[:, :, lo:hi])
            if j == 0:
                ctl = pool.tile([128, SEG, 3], dt.float32, tag="clsin", bufs=2,
                                name="ctl")
                cpl = pool.tile([128, SEG, 3], dt.float32, tag="clsin", bufs=2,
                                name="cpl")
                nc.sync.dma_start(ctl[:], tcl[:])
                nc.sync.dma_start(cpl[:], pcl[:])

        CMAX = max(LW)
        done_class = False
        for j in range(NL):
            w = LW[j]
            xt = tas[j][:, :, 0 : w + 4]
            xp = pas[j][:, :, 0 : w + 4]
            dbf = pool.tile([128, SEG, CMAX + 4], DT16, tag="dbf", bufs=3,
                            name=f"dbf{j}")
            nc.vector.tensor_sub(dbf[:, :, 0 : w + 4], xt, xp)
            u = pool.tile([128, SEG, CMAX], DT16, tag="u", bufs=3, name=f"u{j}")
            nc.vector.tensor_add(u[:, :, 0:w], dbf[:, :, 1 : w + 1],
                                 dbf[:, :, 3 : w + 3])
            v = pool.tile([128, SEG, CMAX], DT16, tag="v", bufs=3, name=f"v{j}")
            nc.vector.tensor_add(v[:, :, 0:w], dbf[:, :, 0:w],
                                 dbf[:, :, 4 : w + 4])

            if j < N_PE:
                for s in range(SEG):
                    psum = ppool.tile([128, CMAX], dt.float32, tag="ps",
                                      name=f"ps{j}_{s}")
                    nch = (w + CH - 1) // CH
                    for c in range(nch):
                        c0, c1 = c * CH, min((c + 1) * CH, w)
                        nc.tensor.matmul(psum[:, c0:c1], id14,
                                         dbf[:, s, 2 + c0 : 2 + c1],
                                         start=True, stop=False)
                    for c in range(nch):
                        c0, c1 = c * CH, min((c + 1) * CH, w)
                        nc.tensor.matmul(psum[:, c0:c1], id2, u[:, s, c0:c1],
                                         start=False, stop=False)
                    for c in range(nch):
                        c0, c1 = c * CH, min((c + 1) * CH, w)
                        nc.tensor.matmul(psum[:, c0:c1], id1, v[:, s, c0:c1],
                                         start=False, stop=True)
                    sq = pool.tile([128, CMAX], DT16, tag="sq", bufs=3,
                                   name=f"sq{j}_{s}")
                    col = j * SEG + s
                    nc.scalar.activation(
                        sq[:, 0:w], psum[:, 0:w], AF.Square,
                        accum_out=accA[:, col : col + 1],
                    )
            else:
                # tail: single-engine DVE chain over both segments
                r2 = pool.tile([128, SEG, 256], DT16, tag="r2t", bufs=2,
                               name=f"r2t{j}")
                nc.vector.scalar_tensor_tensor(
                    r2[:, :, 0:w], u[:, :, 0:w], 2.0, v[:, :, 0:w],
                    OP.mult, OP.add)
                st = pool.tile([128, SEG, 256], DT16, tag="st", bufs=2,
                               name=f"st{j}")
                nc.vector.scalar_tensor_tensor(
                    st[:, :, 0:w], dbf[:, :, 2 : w + 2], 14.0, r2[:, :, 0:w],
                    OP.mult, OP.add)
                sqt = pool.tile([128, SEG, 256], DT16, tag="sqt", bufs=2,
                                name=f"sqt{j}")
                col = j - N_PE
                nc.vector.scalar_tensor_tensor(
                    sqt[:, :, 0:w], st[:, :, 0:w], 1.0, st[:, :, 0:w],
                    OP.bypass, OP.mult,
                    accum_out=accT[:, col : col + 1],
                )

            if not done_class:
                done_class = True
                cd = pool.tile([128, SEG, 3], dt.float32, tag="clsmid", bufs=2,
                               name="cd")
                nc.vector.tensor_sub(cd[:], ctl[:], cpl[:])
                cj = pool.tile([128, SEG, 3], dt.float32, tag="clsmid", bufs=2,
                               name="cj")
                nc.scalar.activation(
                    cj[:], cd[:], AF.Square,
                    accum_out=accC[:, 0:1],
                )

        npe_cols = N_PE * SEG
        nc.sync.dma_start(out[:, npe_cols : npe_cols + (NL - N_PE)], accT[:])
        nc.sync.dma_start(out[:, NACC - 1 : NACC], accC[:])
        nc.sync.dma_start(out[:, 0:npe_cols], accA[:])

    nc.finalize()
    return nc


_NC = None
last_result = None  # BassKernelResults of the most recent run (for test harness)


def kernel(target_angle, pred_angle, target_class, pred_class):
    global _NC, last_result
    if _NC is None:
        _NC = build_nc()

    ta16 = np.asarray(target_angle, dtype=np.float16)
    pa16 = np.asarray(pred_angle, dtype=np.float16)
    tc32 = np.asarray(target_class, dtype=np.float32)
    pc32 = np.asarray(pred_class, dtype=np.float32)

    in_maps = []
    for c in range(N_CORES):
        r = slice(c * RPC, (c + 1) * RPC)
        in_maps.append(
            {
                "target_angle": np.ascontiguousarray(ta16[r]).reshape(128, SEG, T),
                "pred_angle": np.ascontiguousarray(pa16[r]).reshape(128, SEG, T),
                "target_class": np.ascontiguousarray(tc32[r]).reshape(128, SEG, 3),
                "pred_class": np.ascontiguousarray(pc32[r]).reshape(128, SEG, 3),
            }
        )

    last_result = run_bass_kernel_spmd(
        _NC,
        in_maps,
        core_ids=list(range(N_CORES)),
        trace=bool(os.environ.get("BASS_TRACE")),
    )

    angle = 0.0
    cls = 0.0
    na = NACC - 1
    for r in last_result.results:
        o = np.asarray(r["out"], dtype=np.float64)
        angle += o[:, 0:na].sum()
        cls += o[:, na:NACC].sum()

    val = 0.8 * (W4 * W4) * angle + 0.2 * cls
    return np.array(val, dtype=np.float32)


# revision 14
# speedup vs baseline: 1.1907x; 1.1907x over previous
"""Trainium2 Bass kernel for nn_D_loss_67551245631962.

Computes: 0.8 * sum(WMA5(target_angle - pred_angle)^2) + 0.2 * sum((target_class - pred_class)^2)
where WMA5 is a 5-tap [0.05, 0.1, 0.7, 0.1, 0.05] correlation with 2-zero padding per side.

Strategy (pure data parallelism over batch dim B=2048 across 8 cores, 256 rows/core):
  - Inputs cast to fp16 on the host (same numerics as the original on-chip
    cast-DMA pipeline, ~1e-5 end-to-end) -> per-core HBM read 8.4 MB.
  - Row-group merge: the 256 rows/core are laid out [128 partitions, 2 row
    segments] (DRAM tensor declared [128, 2, T]; partition p holds rows
    2p, 2p+1). Every DVE op covers BOTH segments as one 3D-AP instruction,
    halving instruction/semaphore overhead vs the 2x128-group layout.
  - All loads on the SP HWDGE ring (measured 300+ GB/s interleaved; keeps
    ACT free of emissions, GpSimd free). Chunks [512,2048,2048,2048,1024,
    256,256]: small lead (compute starts ~5us in), small tail.
  - s = 14*d2 + 2*u + v (u = d1+d3, v = d0+d4) = wma/0.05, exact 5 taps.
    DVE: dbf = ta-pa, u, v (GpSimd compute is NOT used: concurrent Q7
    tensor ops degrade DVE throughput ~4x - measured).
    j=0..4 (93.75%): PE psum = 14I@d2 + 2I@u + I@v per (chunk, seg);
      ACT squares psum with accum_out.
    j=5,6 (the tail): all-DVE STT chain r2=2u+v, s=14*d2+r2,
      sq=s*s with accum_out - no cross-engine hops after the last load.
  - Halo memsets + identity stationaries built on GpSimd before loads.
  - Host sums the 8 cores' [128, 16] partials in float64, scales by
    0.8*0.05^2 (angle) and 0.2 (class).
  Engine budget/core (measured rates): DVE ~42us, PE ~43us, ACT ~27us,
  sync ~19us, DMA ~24us. Bottleneck: DVE/PE ~43us.
"""

import os
import sys

os.environ.setdefault("TILE_SCHEDULER", "asap")

for _p in ("/opt/trn_rl_repo",):
    if os.path.isdir(_p) and _p not in sys.path:
        sys.path.insert(0, _p)

from contextlib import ExitStack

import numpy as np

import concourse.bass as bass
import concourse.tile as tile
from concourse import bacc, mybir
from concourse.bass_utils import run_bass_kernel_spmd

N_CORES = 8
B, T = 2048, 8192
RPC = B // N_CORES  # rows per core = 256
SEG = 2             # row segments per partition (rows 2p, 2p+1)

LW = [512, 2048, 2048, 2048, 1024, 256, 256]
assert sum(LW) == T
LSTART = [sum(LW[:j]) for j in range(len(LW))]
NL = len(LW)
N_PE = 5      # chunks 0..4 -> PE path; 5,6 -> all-DVE tail
NACC = N_PE * SEG + (NL - N_PE) + 1  # PE cols + tail cols + class col = 13
CH = 512

W4 = 0.05
DT16 = mybir.dt.float16


def build_nc():
    nc = bacc.Bacc("TRN2")
    dt = mybir.dt
    ta = nc.dram_tensor("target_angle", [128, SEG, T], DT16, kind="ExternalInput")
    pa = nc.dram_tensor("pred_angle", [128, SEG, T], DT16, kind="ExternalInput")
    tcl = nc.dram_tensor("target_class", [128, SEG, 3], dt.float32, kind="ExternalInput")
    pcl = nc.dram_tensor("pred_class", [128, SEG, 3], dt.float32, kind="ExternalInput")
    out = nc.dram_tensor("out", [128, NACC], dt.float32, kind="ExternalOutput")

    AF = mybir.ActivationFunctionType
    OP = mybir.AluOpType

    def lgeom(j):
        c0, w = LSTART[j], LW[j]
        lo, hi = c0 - 2, c0 + w + 2
        dst_lo, dst_hi = 0, w + 4
        if lo < 0:
            dst_lo, lo = 2, 0
        if hi > T:
            dst_hi, hi = w + 2, T
        return lo, hi, dst_lo, dst_hi

    with tile.TileContext(nc) as tc, ExitStack() as ctx:
        pool = ctx.enter_context(tc.tile_pool(name="main", bufs=1))
        ppool = ctx.enter_context(tc.tile_pool(name="ps", bufs=2, space="PSUM"))

        accums = pool.tile([128, NACC], dt.float32, tag="acc", bufs=1)

        def make_diag(scale, name):
            m = pool.tile([128, 128], DT16, tag="diag", bufs=6, name=f"m_{name}")
            nc.gpsimd.memset(m[:], scale)
            s = pool.tile([128, 128], DT16, tag="diag", bufs=6, name=f"id_{name}")
            nc.gpsimd.affine_select(
                s[:], m[:], [[1, 128]], OP.is_equal, 0.0,
                base=0, channel_multiplier=-1,
            )
            return s

        id14 = make_diag(14.0, "w14")
        id2 = make_diag(2.0, "w2")
        id1 = make_diag(1.0, "w1")

        tas = [None] * NL
        pas = [None] * NL
        for j in range(NL):
            wid = LW[j] + 4
            tas[j] = pool.tile([128, SEG, wid], DT16, tag=f"ta{j}", bufs=1,
                               name=f"ta_{j}")
            pas[j] = pool.tile([128, SEG, wid], DT16, tag=f"pa{j}", bufs=1,
                               name=f"pa_{j}")

        # halo zeros on GpSimd (it is idle before loads; no DVE interference)
        wlast = LW[NL - 1]
        for tl in (tas[0], pas[0]):
            nc.gpsimd.memset(tl[:, :, 0:2], 0.0)
        for tl in (tas[NL - 1], pas[NL - 1]):
            nc.gpsimd.memset(tl[:, :, wlast + 2 : wlast + 4], 0.0)

        # all loads on the SP HWDGE ring; ta then pa per chunk; small lead,
        # small tail
        ctl = cpl = None
        for j in range(NL):
            lo, hi, dst_lo, dst_hi = lgeom(j)
            nc.sync.dma_start(tas[j][:, :, dst_lo:dst_hi], ta[:, :, lo:hi])
            nc.sync.dma_start(pas[j][:, :, dst_lo:dst_hi], pa[:, :, lo:hi])
            if j == 0:
                ctl = pool.tile([128, SEG, 3], dt.float32, tag="clsin", bufs=2,
                                name="ctl")
                cpl = pool.tile([128, SEG, 3], dt.float32, tag="clsin", bufs=2,
                                name="cpl")
                nc.sync.dma_start(ctl[:], tcl[:])
                nc.sync.dma_start(cpl[:], pcl[:])

        CMAX = max(LW)
        done_class = False
        for j in range(NL):
            w = LW[j]
            xt = tas[j][:, :, 0 : w + 4]
            xp = pas[j][:, :, 0 : w + 4]
            dbf = pool.tile([128, SEG, CMAX + 4], DT16, tag="dbf", bufs=3,
                            name=f"dbf{j}")
            nc.vector.tensor_sub(dbf[:, :, 0 : w + 4], xt, xp)
            u = pool.tile([128, SEG, CMAX], DT16, tag="u", bufs=3, name=f"u{j}")
            nc.vector.tensor_add(u[:, :, 0:w], dbf[:, :, 1 : w + 1],
                                 dbf[:, :, 3 : w + 3])
            v = pool.tile([128, SEG, CMAX], DT16, tag="v", bufs=3, name=f"v{j}")
            nc.vector.tensor_add(v[:, :, 0:w], dbf[:, :, 0:w],
                                 dbf[:, :, 4 : w + 4])

            if j < N_PE:
                for s in range(SEG):
                    psum = ppool.tile([128, CMAX], dt.float32, tag="ps",
                                      name=f"ps{j}_{s}")
                    nch = (w + CH - 1) // CH
                    for c in range(nch):
                        c0, c1 = c * CH, min((c + 1) * CH, w)
                        nc.tensor.matmul(psum[:, c0:c1], id14,
                                         dbf[:, s, 2 + c0 : 2 + c1],
                                         start=True, stop=False)
                    for c in range(nch):
                        c0, c1 = c * CH, min((c + 1) * CH, w)
                        nc.tensor.matmul(psum[:, c0:c1], id2, u[:, s, c0:c1],
                                         start=False, stop=False)
                    for c in range(nch):
                        c0, c1 = c * CH, min((c + 1) * CH, w)
                        nc.tensor.matmul(psum[:, c0:c1], id1, v[:, s, c0:c1],
                                         start=False, stop=True)
                    sq = pool.tile([128, CMAX], DT16, tag="sq", bufs=3,
                                   name=f"sq{j}_{s}")
                    col = j * SEG + s
                    nc.scalar.activation(
                        sq[:, 0:w], psum[:, 0:w], AF.Square,
                        accum_out=accums[:, col : col + 1],
                    )
            else:
                # tail: single-engine DVE chain over both segments
                r2 = pool.tile([128, SEG, 256], DT16, tag="r2t", bufs=2,
                               name=f"r2t{j}")
                nc.vector.scalar_tensor_tensor(
                    r2[:, :, 0:w], u[:, :, 0:w], 2.0, v[:, :, 0:w],
                    OP.mult, OP.add)
                st = pool.tile([128, SEG, 256], DT16, tag="st", bufs=2,
                               name=f"st{j}")
                nc.vector.scalar_tensor_tensor(
                    st[:, :, 0:w], dbf[:, :, 2 : w + 2], 14.0, r2[:, :, 0:w],
                    OP.mult, OP.add)
                sqt = pool.tile([128, SEG, 256], DT16, tag="sqt", bufs=2,
                                name=f"sqt{j}")
                col = N_PE * SEG + (j - N_PE)
                nc.vector.scalar_tensor_tensor(
                    sqt[:, :, 0:w], st[:, :, 0:w], 1.0, st[:, :, 0:w],
                    OP.bypass, OP.mult,
                    accum_out=accums[:, col : col + 1],
                )

            if not done_class:
                done_class = True
                cd = pool.tile([128, SEG, 3], dt.float32, tag="clsmid", bufs=2,
                               name="cd")
                nc.vector.tensor_sub(cd[:], ctl[:], cpl[:])
                cj = pool.tile([128, SEG, 3], dt.float32, tag="clsmid", bufs=2,
                               name="cj")
                ccol = NACC - 1
                nc.scalar.activation(
                    cj[:], cd[:], AF.Square,
                    accum_out=accums[:, ccol : ccol + 1],
                )

        nc.sync.dma_start(out[:], accums[:])

    nc.finalize()
    return nc


_NC = None
last_result = None  # BassKernelResults of the most recent run (for test harness)


def kernel(target_angle, pred_angle, target_class, pred_class):
    global _NC, last_result
    if _NC is None:
        _NC = build_nc()

    ta16 = np.asarray(target_angle, dtype=np.float16)
    pa16 = np.asarray(pred_angle, dtype=np.float16)
    tc32 = np.asarray(target_class, dtype=np.float32)
    pc32 = np.asarray(pred_class, dtype=np.float32)

    in_maps = []
    for c in range(N_CORES):
        r = slice(c * RPC, (c + 1) * RPC)
        in_maps.append(
            {
                "target_angle": np.ascontiguousarray(ta16[r]).reshape(128, SEG, T),
                "pred_angle": np.ascontiguousarray(pa16[r]).reshape(128, SEG, T),
                "target_class": np.ascontiguousarray(tc32[r]).reshape(128, SEG, 3),
                "pred_class": np.ascontiguousarray(pc32[r]).reshape(128, SEG, 3),
            }
        )

    last_result = run_bass_kernel_spmd(
        _NC,
        in_maps,
        core_ids=list(range(N_CORES)),
        trace=bool(os.environ.get("BASS_TRACE")),
    )

    angle = 0.0
    cls = 0.0
    na = NACC - 1
    for r in last_result.results:
        o = np.asarray(r["out"], dtype=np.float64)
        angle += o[:, 0:na].sum()
        cls += o[:, na:NACC].sum()

    val = 0.8 * (W4 * W4) * angle + 0.2 * cls
    return np.array(val, dtype=np.float32)


# revision 15
# speedup vs baseline: 1.2231x; 1.0272x over previous
"""Trainium2 Bass kernel for nn_D_loss_67551245631962.

Computes: 0.8 * sum(WMA5(target_angle - pred_angle)^2) + 0.2 * sum((target_class - pred_class)^2)
where WMA5 is a 5-tap [0.05, 0.1, 0.7, 0.1, 0.05] correlation with 2-zero padding per side.

Strategy (pure data parallelism over batch dim B=2048 across 8 cores, 256 rows/core):
  - Inputs cast to fp16 on the host (same numerics as the original on-chip
    cast-DMA pipeline, ~1e-5 end-to-end) -> per-core HBM read 8.4 MB.
  - Row-group merge: the 256 rows/core are laid out [128 partitions, 2 row
    segments] (DRAM tensor declared [128, 2, T]; partition p holds rows
    2p, 2p+1). Every DVE op covers BOTH segments as one 3D-AP instruction,
    halving instruction/semaphore overhead vs the 2x128-group layout.
  - All loads on the SP HWDGE ring (measured 300+ GB/s interleaved; keeps
    ACT free of emissions, GpSimd free). Chunks [512,2048,2048,2048,1024,
    256,256]: small lead (compute starts ~5us in), small tail.
  - s = 14*d2 + 2*u + v (u = d1+d3, v = d0+d4) = wma/0.05, exact 5 taps.
    DVE: dbf = ta-pa, u, v (GpSimd compute is NOT used: concurrent Q7
    tensor ops degrade DVE throughput ~4x - measured).
    j=0..4 (93.75%): PE psum = 14I@d2 + 2I@u + I@v per (chunk, seg);
      ACT squares psum with accum_out.
    j=5,6 (the tail): all-DVE STT chain r2=2u+v, s=14*d2+r2,
      sq=s*s with accum_out - no cross-engine hops after the last load.
  - Halo memsets + identity stationaries built on GpSimd before loads.
  - Host sums the 8 cores' [128, 16] partials in float64, scales by
    0.8*0.05^2 (angle) and 0.2 (class).
  Engine budget/core (measured rates): DVE ~42us, PE ~43us, ACT ~27us,
  sync ~19us, DMA ~24us. Bottleneck: DVE/PE ~43us.
"""

import os
import sys

os.environ.setdefault("TILE_SCHEDULER", "asap")

for _p in ("/opt/trn_rl_repo",):
    if os.path.isdir(_p) and _p not in sys.path:
        sys.path.insert(0, _p)

from contextlib import ExitStack

import numpy as np

import concourse.bass as bass
import concourse.tile as tile
from concourse import bacc, mybir
from concourse.bass_utils import run_bass_kernel_spmd

N_CORES = 8
B, T = 2048, 8192
RPC = B // N_CORES  # rows per core = 256
SEG = 2             # row segments per partition (rows 2p, 2p+1)

LW = [512, 2048, 2048, 2048, 1024, 512]
assert sum(LW) == T
LSTART = [sum(LW[:j]) for j in range(len(LW))]
NL = len(LW)
N_PE = 5      # chunks 0..4 -> PE path; 5 -> all-DVE tail
NACC = N_PE * SEG + (NL - N_PE) + 1  # PE cols + tail cols + class col = 13
CH = 512

W4 = 0.05
DT16 = mybir.dt.float16


def build_nc():
    nc = bacc.Bacc("TRN2")
    dt = mybir.dt
    ta = nc.dram_tensor("target_angle", [128, SEG, T], DT16, kind="ExternalInput")
    pa = nc.dram_tensor("pred_angle", [128, SEG, T], DT16, kind="ExternalInput")
    tcl = nc.dram_tensor("target_class", [128, SEG, 3], dt.float32, kind="ExternalInput")
    pcl = nc.dram_tensor("pred_class", [128, SEG, 3], dt.float32, kind="ExternalInput")
    out = nc.dram_tensor("out", [128, NACC], dt.float32, kind="ExternalOutput")

    AF = mybir.ActivationFunctionType
    OP = mybir.AluOpType

    def lgeom(j):
        c0, w = LSTART[j], LW[j]
        lo, hi = c0 - 2, c0 + w + 2
        dst_lo, dst_hi = 0, w + 4
        if lo < 0:
            dst_lo, lo = 2, 0
        if hi > T:
            dst_hi, hi = w + 2, T
        return lo, hi, dst_lo, dst_hi

    with tile.TileContext(nc) as tc, ExitStack() as ctx:
        pool = ctx.enter_context(tc.tile_pool(name="main", bufs=1))
        ppool = ctx.enter_context(tc.tile_pool(name="ps", bufs=2, space="PSUM"))

        accums = pool.tile([128, NACC], dt.float32, tag="acc", bufs=1)

        def make_diag(scale, name):
            m = pool.tile([128, 128], DT16, tag="diag", bufs=6, name=f"m_{name}")
            nc.gpsimd.memset(m[:], scale)
            s = pool.tile([128, 128], DT16, tag="diag", bufs=6, name=f"id_{name}")
            nc.gpsimd.affine_select(
                s[:], m[:], [[1, 128]], OP.is_equal, 0.0,
                base=0, channel_multiplier=-1,
            )
            return s

        id14 = make_diag(14.0, "w14")
        id2 = make_diag(2.0, "w2")
        id1 = make_diag(1.0, "w1")

        tas = [None] * NL
        pas = [None] * NL
        for j in range(NL):
            wid = LW[j] + 4
            tas[j] = pool.tile([128, SEG, wid], DT16, tag=f"ta{j}", bufs=1,
                               name=f"ta_{j}")
            pas[j] = pool.tile([128, SEG, wid], DT16, tag=f"pa{j}", bufs=1,
                               name=f"pa_{j}")

        # halo zeros on GpSimd (it is idle before loads; no DVE interference)
        wlast = LW[NL - 1]
        for tl in (tas[0], pas[0]):
            nc.gpsimd.memset(tl[:, :, 0:2], 0.0)
        for tl in (tas[NL - 1], pas[NL - 1]):
            nc.gpsimd.memset(tl[:, :, wlast + 2 : wlast + 4], 0.0)

        # all loads on the SP HWDGE ring; ta then pa per chunk; small lead,
        # small tail
        ctl = cpl = None
        for j in range(NL):
            lo, hi, dst_lo, dst_hi = lgeom(j)
            nc.sync.dma_start(tas[j][:, :, dst_lo:dst_hi], ta[:, :, lo:hi])
            nc.sync.dma_start(pas[j][:, :, dst_lo:dst_hi], pa[:, :, lo:hi])
            if j == 0:
                ctl = pool.tile([128, SEG, 3], dt.float32, tag="clsin", bufs=2,
                                name="ctl")
                cpl = pool.tile([128, SEG, 3], dt.float32, tag="clsin", bufs=2,
                                name="cpl")
                nc.sync.dma_start(ctl[:], tcl[:])
                nc.sync.dma_start(cpl[:], pcl[:])

        CMAX = max(LW)
        done_class = False
        for j in range(NL):
            w = LW[j]
            xt = tas[j][:, :, 0 : w + 4]
            xp = pas[j][:, :, 0 : w + 4]
            dbf = pool.tile([128, SEG, CMAX + 4], DT16, tag="dbf", bufs=3,
                            name=f"dbf{j}")
            nc.vector.tensor_sub(dbf[:, :, 0 : w + 4], xt, xp)
            u = pool.tile([128, SEG, CMAX], DT16, tag="u", bufs=3, name=f"u{j}")
            nc.vector.tensor_add(u[:, :, 0:w], dbf[:, :, 1 : w + 1],
                                 dbf[:, :, 3 : w + 3])
            v = pool.tile([128, SEG, CMAX], DT16, tag="v", bufs=3, name=f"v{j}")
            nc.vector.tensor_add(v[:, :, 0:w], dbf[:, :, 0:w],
                                 dbf[:, :, 4 : w + 4])

            if j < N_PE:
                for s in range(SEG):
                    psum = ppool.tile([128, CMAX], dt.float32, tag="ps",
                                      name=f"ps{j}_{s}")
                    nch = (w + CH - 1) // CH
                    for c in range(nch):
                        c0, c1 = c * CH, min((c + 1) * CH, w)
                        nc.tensor.matmul(psum[:, c0:c1], id14,
                                         dbf[:, s, 2 + c0 : 2 + c1],
                                         start=True, stop=False)
                    for c in range(nch):
                        c0, c1 = c * CH, min((c + 1) * CH, w)
                        nc.tensor.matmul(psum[:, c0:c1], id2, u[:, s, c0:c1],
                                         start=False, stop=False)
                    for c in range(nch):
                        c0, c1 = c * CH, min((c + 1) * CH, w)
                        nc.tensor.matmul(psum[:, c0:c1], id1, v[:, s, c0:c1],
                                         start=False, stop=True)
                    sq = pool.tile([128, CMAX], DT16, tag="sq", bufs=3,
                                   name=f"sq{j}_{s}")
                    col = j * SEG + s
                    nc.scalar.activation(
                        sq[:, 0:w], psum[:, 0:w], AF.Square,
                        accum_out=accums[:, col : col + 1],
                    )
            else:
                # tail: single-engine DVE chain over both segments
                r2 = pool.tile([128, SEG, 512], DT16, tag="r2t", bufs=2,
                               name=f"r2t{j}")
                nc.vector.scalar_tensor_tensor(
                    r2[:, :, 0:w], u[:, :, 0:w], 2.0, v[:, :, 0:w],
                    OP.mult, OP.add)
                st = pool.tile([128, SEG, 512], DT16, tag="st", bufs=2,
                               name=f"st{j}")
                nc.vector.scalar_tensor_tensor(
                    st[:, :, 0:w], dbf[:, :, 2 : w + 2], 14.0, r2[:, :, 0:w],
                    OP.mult, OP.add)
                sqt = pool.tile([128, SEG, 512], DT16, tag="sqt", bufs=2,
                                name=f"sqt{j}")
                col = N_PE * SEG + (j - N_PE)
                nc.vector.scalar_tensor_tensor(
                    sqt[:, :, 0:w], st[:, :, 0:w], 1.0, st[:, :, 0:w],
                    OP.bypass, OP.mult,
                    accum_out=accums[:, col : col + 1],
                )

            if not done_class:
                done_class = True
                cd = pool.tile([128, SEG, 3], dt.float32, tag="clsmid", bufs=2,
                               name="cd")
                nc.vector.tensor_sub(cd[:], ctl[:], cpl[:])
                cj = pool.tile([128, SEG, 3], dt.float32, tag="clsmid", bufs=2,
                               name="cj")
                ccol = NACC - 1
                nc.scalar.activation(
                    cj[:], cd[:], AF.Square,
                    accum_out=accums[:, ccol : ccol + 1],
                )

        nc.sync.dma_start(out[:], accums[:])

    nc.finalize()
    return nc


_NC = None
last_result = None  # BassKernelResults of the most recent run (for test harness)


def kernel(target_angle, pred_angle, target_class, pred_class):
    global _NC, last_result
    if _NC is None:
        _NC = build_nc()

    ta16 = np.asarray(target_angle, dtype=np.float16)
    pa16 = np.asarray(pred_angle, dtype=np.float16)
    tc32 = np.asarray(target_class, dtype=np.float32)
    pc32 = np.asarray(pred_class, dtype=np.float32)

    in_maps = []
    for c in range(N_CORES):
        r = slice(c * RPC, (c + 1) * RPC)
        in_maps.append(
            {
                "target_angle": np.ascontiguousarray(ta16[r]).reshape(128, SEG, T),
                "pred_angle": np.ascontiguousarray(pa16[r]).reshape(128, SEG, T),
                "target_class": np.ascontiguousarray(tc32[r]).reshape(128, SEG, 3),
                "pred_class": np.ascontiguousarray(pc32[r]).reshape(128, SEG, 3),
            }
        )

    last_result = run_bass_kernel_spmd(
        _NC,
        in_maps,
        core_ids=list(range(N_CORES)),
        trace=bool(os.environ.get("BASS_TRACE")),
    )

    angle = 0.0
    cls = 0.0
    na = NACC - 1
    for r in last_result.results:
        o = np.asarray(r["out"], dtype=np.float64)
        angle += o[:, 0:na].sum()
        cls += o[:, na:NACC].sum()

    val = 0.8 * (W4 * W4) * angle + 0.2 * cls
    return np.array(val, dtype=np.float32)
